# revision 45
# baseline (speedup 1.0000x reference)
"""Trainium2 Bass kernel for nn_FRNNPathB (scatter_memory).

Strategy
--------
Data-parallel over batch B=8 across 8 NeuronCores (one batch element per core).

Key algorithmic observation: after the sequential mode scan, every downstream
quantity (mem, bank attention read, RMS norm, output projection) depends on the
token only through its mode index j in [0, 256). So the whole post-scan network
collapses to a per-mode table y_table[256, DOUT] computed once per core, and
y[s] = y_table[j_s] is a row gather. The bank attention (B*S*BANK*DM work in the
reference) is computed for 256 rows instead of 16384.

Per-core pipeline:
  Phase A (table): l2-normalize M rows and bank_keys, scores = qn @ kn^T * 4,
    attn = exp(scores) (cosine scores are bounded by +-4, no max-sub needed),
    bank = (attn @ (used*vals)) / (attn @ used), r = M + bank, RMS-norm with g
    folded into Wrd rows, y_table = rn @ Wrd + b  -> DRAM scratch.
  Phase B (logits): h^T = relu(Wtr^T x^T + b) and logits = h @ Wms + b in true
    fp32 on the PE (the argmax/stickiness decisions need fp32-level accuracy;
    measured min decision margin is ~2e-6 while fp32 cross-impl noise is ~3e-7).
  Scan: logits reduce to (max v_t, argmax a_t, staymask_t = lg_t >= v_t - 0.1).
    j_t = j_{t-1} if staymask[t, j_{t-1}] else a_t.  We precompute, in int16,
    sel[t, k] = 256*((t+1)%64) + (k if staymask[t,k] else a_t), sliced into 32
    tensors of 64 steps ([128, 128] i16, linear offset 256*(t%64)+j), so the
    scan is a pure pointer chase: one dependent reg_load per step on the SP
    sequencer (plus a fire-and-forget reg_save of the trajectory).
  Outputs: modes[t] = (iota == j_t), y rows gathered from y_table via dma_gather.
"""
import sys

sys.path.insert(0, "/opt/trn_rl_repo")

import numpy as np
from contextlib import ExitStack

import concourse.bacc as bacc
import concourse.bass as bass
import concourse.mybir as mybir
import concourse.tile as tile
from concourse import library_config
from concourse.tile_rust import add_dep_helper

B, S, DIN, H, K, DM, DOUT, BANK = 8, 2048, 1024, 2048, 256, 512, 1024, 4096
STICKINESS = 0.1
BANK_SCALE = 4.0
RMS_EPS = 1e-6

NCORES = 8
SGRP = 512                     # S columns per matmul-1 group
NGRP = S // SGRP               # 2
CHUNK = 128                    # logits chunk (= partition count per S tile)
NCHUNK = S // CHUNK            # 16
SCH = 64                       # scan steps per scan tensor
NSCH = S // SCH                # 32
F32 = mybir.dt.float32
F32R = mybir.dt.float32r
I32 = mybir.dt.int32
I16 = mybir.dt.int16
U32 = mybir.dt.uint32
AF = mybir.ActivationFunctionType
ALU = mybir.AluOpType


def build_kernel(debug_outputs: bool = False):
    """Build the single-core Bass program (SPMD across 8 cores)."""
    nc = bacc.Bacc("TRN2", target_bir_lowering=False, debug=False)

    # ---- external inputs (per core) ----
    x = nc.dram_tensor("x", [S, DIN], F32, kind="ExternalInput").ap()
    prev_mode = nc.dram_tensor("prev_mode", [1, K], F32, kind="ExternalInput").ap()
    wtr = nc.dram_tensor("Wtr_w", [DIN, H], F32, kind="ExternalInput").ap()
    wtrb = nc.dram_tensor("Wtr_b", [H], F32, kind="ExternalInput").ap()
    wms = nc.dram_tensor("Wms_w", [H, K], F32, kind="ExternalInput").ap()
    wmsb = nc.dram_tensor("Wms_b", [K], F32, kind="ExternalInput").ap()
    M_in = nc.dram_tensor("M", [K, DM], F32, kind="ExternalInput").ap()
    g_in = nc.dram_tensor("g", [DM], F32, kind="ExternalInput").ap()
    wrd = nc.dram_tensor("Wrd_w", [DM, DOUT], F32, kind="ExternalInput").ap()
    wrdb = nc.dram_tensor("Wrd_b", [DOUT], F32, kind="ExternalInput").ap()
    bkeys = nc.dram_tensor("bank_keys", [BANK, DM], F32, kind="ExternalInput").ap()
    bvals = nc.dram_tensor("bank_vals", [BANK, DM], F32, kind="ExternalInput").ap()
    bused = nc.dram_tensor("bank_used", [BANK], F32, kind="ExternalInput").ap()
    # host-provided constants (shape-derived only)
    iota_f = nc.dram_tensor("iota_f", [128, K], F32, kind="ExternalInput").ap()
    basecol = nc.dram_tensor("basecol", [128, 1], F32, kind="ExternalInput").ap()
    iden = nc.dram_tensor("iden", [128, 128], F32, kind="ExternalInput").ap()
    iden256 = nc.dram_tensor("iden256", [K, K], F32, kind="ExternalInput").ap()

    # ---- external outputs (per core) ----
    y_out = nc.dram_tensor("y", [S, DOUT], F32, kind="ExternalOutput").ap()
    modes_out = nc.dram_tensor("modes", [S, K], F32, kind="ExternalOutput").ap()
    if debug_outputs:
        dbg_traj = nc.dram_tensor("dbg_traj", [128, NCHUNK], I32,
                                  kind="ExternalOutput").ap()
        dbg_ytab = nc.dram_tensor("dbg_ytab", [K, DOUT], F32, kind="ExternalOutput").ap()
        dbg_sel = nc.dram_tensor("dbg_sel", [128, NCHUNK * K], I16,
                                 kind="ExternalOutput").ap()
        dbg_jin = nc.dram_tensor("dbg_jin", [1, 1], I32, kind="ExternalOutput").ap()

    # DRAM scratch: per-mode [y_table | I256] rows for the output gather
    ytab_dram = nc.dram_tensor("ytab_scratch", [K, DOUT + K], F32).ap()
    trajd = nc.dram_tensor("traj_scratch", [1, S], I32).ap()

    with tile.TileContext(nc) as tc, ExitStack() as ctx:
        sync, vec, act, pe, gp = nc.sync, nc.vector, nc.scalar, nc.tensor, nc.gpsimd

        lib_inst = gp.load_library(library_config.mlp)

        # ---------------- small persistent constants ----------------
        iota_sb = nc.alloc_sbuf_tensor("iota_sb", [128, K], F32)
        sync.dma_start(iota_sb.ap(), iota_f)
        base_sb = nc.alloc_sbuf_tensor("base_sb", [128, 1], F32)
        sync.dma_start(base_sb.ap(), basecol)
        iden_sb = nc.alloc_sbuf_tensor("iden_sb", [128, 128], F32)
        sync.dma_start(iden_sb.ap(), iden)
        ones_sb = nc.alloc_sbuf_tensor("ones_sb", [1, K], F32)
        vec.memset(ones_sb.ap(), 1.0)

        # all raw (non-pool) SBUF tensors up front, so Tile pools never
        # overlap their address range
        sel_nat = nc.alloc_sbuf_tensor("sel_nat", [128, NCHUNK * K], I16)
        # ping-pong single-partition scan buffers: 64 steps each; the dynamic
        # reg_load offset must stay within one partition row
        scanbuf = [nc.alloc_sbuf_tensor(f"scanbuf{b}", [1, SCH * K], I16)
                   for b in range(2)]
        traj1 = nc.alloc_sbuf_tensor("traj1", [1, S], I32)
        trajB = nc.alloc_sbuf_tensor("trajB", [128, S // 128], I32)
        pm_sb = nc.alloc_sbuf_tensor("pm_sb", [1, K], F32)
        pmv = nc.alloc_sbuf_tensor("pmv", [1, 8], F32)
        pmi = nc.alloc_sbuf_tensor("pmi", [1, 8], U32)
        jin = nc.alloc_sbuf_tensor("jin", [1, 1], I32)
        jv = nc.alloc_sbuf_tensor("jv", [128, S // 128], I32)
        jfB = nc.alloc_sbuf_tensor("jfB", [128, S // 128], F32)
        idx16 = nc.alloc_sbuf_tensor("idx16", [16, 128], I16)
        idx_sb = nc.alloc_sbuf_tensor("idx_sb", [128, S // 16], I16)

        # =====================================================================
        # Phase A: the 256-row mode table
        # =====================================================================
        with tc.tile_pool(name="pA_small", bufs=2) as pA_small, \
             tc.tile_pool(name="pA_keep", bufs=1) as pA, \
             tc.tile_pool(name="psA_t", bufs=2, space="PSUM") as psA_t, \
             tc.tile_pool(name="psA_m", bufs=2, space="PSUM") as psA_m, \
             tc.tile_pool(name="psA_d", bufs=2, space="PSUM") as psA_d:

            def newton_rsqrt(col_out, col_in, n):
                """1/sqrt via ACT Sqrt + DVE reciprocal + one Newton polish."""
                s0 = pA_small.tile([128, n], F32, tag="nr_s0")
                act.activation(s0[:], col_in, AF.Sqrt)
                y0 = pA_small.tile([128, n], F32, tag="nr_y0")
                vec.reciprocal(y0[:], s0[:])
                t1 = pA_small.tile([128, n], F32, tag="nr_t1")
                vec.tensor_tensor(t1[:], y0[:], s0[:], ALU.mult)
                vec.tensor_scalar(t1[:], t1[:], -1.0, 2.0, ALU.mult, ALU.add)
                vec.tensor_tensor(col_out, y0[:], t1[:], ALU.mult)

            # --- M rows -> qn -> qn^T ---
            m_sb = pA.tile([128, K // 128, DM], F32, tag="m_sb")
            sync.dma_start(m_sb[:], M_in.rearrange("(c p) d -> p c d", p=128))
            msq = pA_small.tile([128, K // 128], F32, tag="msq")
            scr = pA_small.tile([128, DM], F32, tag="scrA")
            for c in range(K // 128):
                act.activation(scr[:], m_sb[:, c, :], AF.Square,
                               accum_out=msq[:, c:c + 1])
            mrsq = pA_small.tile([128, K // 128], F32, tag="mrsq")
            newton_rsqrt(mrsq[:], msq[:], K // 128)
            qnT = pA.tile([128, DM // 128, K], F32, tag="qnT")
            with tc.tile_pool(name="pA_qn", bufs=1) as pA_qn:
                qn_sb = pA_qn.tile([128, K // 128, DM], F32, tag="qn_sb")
                for c in range(K // 128):
                    vec.tensor_scalar_mul(qn_sb[:, c, :], m_sb[:, c, :],
                                          mrsq[:, c:c + 1])
                for c in range(K // 128):
                    for dc in range(DM // 128):
                        pt = psA_t.tile([128, 128], F32, tag="ptA")
                        pe.transpose(pt[:], qn_sb[:, c, dc * 128:(dc + 1) * 128],
                                     iden_sb.ap())
                        act.copy(qnT[:, dc, c * 128:(c + 1) * 128], pt[:])

            # --- bank_keys streamed -> kn^T; attn^T = exp(4 qn kn^T)^T built
            #     strip-by-strip (the full [K, BANK] attn is never materialized)
            pA_at = tc.alloc_tile_pool(name="pA_at", bufs=1)
            attnT = pA_at.tile([128, BANK // 128, K], F32, tag="attnT")
            four_sb = pA_small.tile([128, 1], F32, tag="four")
            vec.memset(four_sb[:], BANK_SCALE)
            HB = BANK // 2
            for half in range(2):
                with tc.tile_pool(name=f"pA_knT{half}", bufs=1) as pA_knT, \
                     tc.tile_pool(name=f"pA_bks{half}", bufs=3) as pA_bks:
                    knT = pA_knT.tile([128, DM // 128, HB], F32, tag="knT")
                    for c in range(HB // 128):
                        gc = half * (HB // 128) + c
                        bkc = pA_bks.tile([128, DM], F32, tag="bkc")
                        sync.dma_start(bkc[:], bkeys[gc * 128:(gc + 1) * 128, :])
                        ksq = pA_small.tile([128, 1], F32, tag="ksq")
                        act.activation(scr[:], bkc[:], AF.Square, accum_out=ksq[:])
                        krs = pA_small.tile([128, 1], F32, tag="krs")
                        newton_rsqrt(krs[:], ksq[:], 1)
                        vec.tensor_scalar_mul(bkc[:], bkc[:], krs[:])
                        for dc in range(DM // 128):
                            pt = psA_t.tile([128, 128], F32, tag="ptA")
                            pe.transpose(pt[:], bkc[:, dc * 128:(dc + 1) * 128],
                                         iden_sb.ap())
                            act.copy(knT[:, dc, c * 128:(c + 1) * 128], pt[:])

                    for mc in range(K // 128):
                        for nb in range(HB // 512):
                            ps = psA_m.tile([128, 512], F32, tag="psAm")
                            for dc in range(DM // 128):
                                pe.matmul(
                                    ps[:],
                                    qnT[:, dc, mc * 128:(mc + 1) * 128],
                                    knT[:, dc, nb * 512:(nb + 1) * 512],
                                    start=(dc == 0), stop=(dc == DM // 128 - 1))
                            strip = pA_bks.tile([128, 512], F32, tag="strip")
                            act.activation(strip[:], ps[:], AF.Exp, scale=four_sb[:])
                            for tb in range(4):
                                bc = half * (HB // 128) + nb * 4 + tb
                                pt = psA_t.tile([128, 128], F32, tag="ptA")
                                pe.transpose(pt[:], strip[:, tb * 128:(tb + 1) * 128],
                                             iden_sb.ap())
                                act.copy(attnT[:, bc, mc * 128:(mc + 1) * 128], pt[:])

            # --- bank read streamed over bank_vals; r = M + bank ---
            r_sb = pA.tile([128, K // 128, DM], F32, tag="r_sb")
            with tc.tile_pool(name="pA_bvstrip", bufs=3) as pA_bvs:
                bv_tiles = []
                for bc in range(BANK // 128):
                    bvc = pA_bvs.tile([128, DM + 1], F32, tag="bvc")
                    sync.dma_start(bvc[:, 0:DM], bvals[bc * 128:(bc + 1) * 128, :])
                    sync.dma_start(bvc[:, DM:DM + 1],
                                   bused[bc * 128:(bc + 1) * 128][:, None])
                    vec.tensor_scalar_mul(bvc[:, 0:DM], bvc[:, 0:DM],
                                          bvc[:, DM:DM + 1])
                    bv_tiles.append(bvc)

                for mc in range(K // 128):
                    psn = psA_m.tile([128, 512], F32, tag="psAm")
                    psd = psA_d.tile([128, 8], F32, tag="psAd")
                    for bc in range(BANK // 128):
                        bvc = bv_tiles[bc]
                        last = bc == BANK // 128 - 1
                        pe.matmul(psn[:],
                                  attnT[:, bc, mc * 128:(mc + 1) * 128],
                                  bvc[:, 0:DM],
                                  start=(bc == 0), stop=last)
                        pe.matmul(psd[:, 0:1],
                                  attnT[:, bc, mc * 128:(mc + 1) * 128],
                                  bvc[:, DM:DM + 1],
                                  start=(bc == 0), stop=last)
                    den = pA_small.tile([128, 1], F32, tag="denA")
                    rec = pA_small.tile([128, 1], F32, tag="recA")
                    act.copy(den[:], psd[:, 0:1])
                    vec.reciprocal(rec[:], den[:])
                    t2 = pA_small.tile([128, 1], F32, tag="recT")
                    vec.tensor_tensor(t2[:], rec[:], den[:], ALU.mult)
                    vec.tensor_scalar(t2[:], t2[:], -1.0, 2.0, ALU.mult, ALU.add)
                    vec.tensor_tensor(rec[:], rec[:], t2[:], ALU.mult)
                    bank_t = pA_small.tile([128, DM], F32, tag="bankA")
                    vec.tensor_scalar_mul(bank_t[:], psn[:], rec[:])
                    vec.tensor_tensor(r_sb[:, mc, :], m_sb[:, mc, :], bank_t[:],
                                      ALU.add)

            pA_at.release()

            # --- rms-norm of r; rn^T; y_table ---
            rsq2 = pA_small.tile([128, K // 128], F32, tag="rsq2")
            for mc in range(K // 128):
                act.activation(scr[:], r_sb[:, mc, :], AF.Square,
                               accum_out=rsq2[:, mc:mc + 1])
            vec.tensor_scalar(rsq2[:], rsq2[:], 1.0 / DM, RMS_EPS, ALU.mult, ALU.add)
            rms = pA_small.tile([128, K // 128], F32, tag="rms")
            newton_rsqrt(rms[:], rsq2[:], K // 128)
            for mc in range(K // 128):
                vec.tensor_scalar_mul(r_sb[:, mc, :], r_sb[:, mc, :],
                                      rms[:, mc:mc + 1])
            rnT = pA.tile([128, DM // 128, K], F32, tag="rnT")
            for mc in range(K // 128):
                for dc in range(DM // 128):
                    pt = psA_t.tile([128, 128], F32, tag="ptA")
                    pe.transpose(pt[:], r_sb[:, mc, dc * 128:(dc + 1) * 128],
                                 iden_sb.ap())
                    act.copy(rnT[:, dc, mc * 128:(mc + 1) * 128], pt[:])

            pA_tail = tc.alloc_tile_pool(name="pA_tail", bufs=1)
            wrd_sb = pA_tail.tile([128, DM // 128, DOUT], F32, tag="wrd_sb")
            sync.dma_start(wrd_sb[:], wrd.rearrange("(c p) d -> p c d", p=128))
            g_sb = pA_small.tile([128, DM // 128], F32, tag="g_sb")
            sync.dma_start(g_sb[:], g_in.rearrange("(c p) -> p c", p=128))
            for dc in range(DM // 128):
                vec.tensor_scalar_mul(wrd_sb[:, dc, :], wrd_sb[:, dc, :],
                                      g_sb[:, dc:dc + 1])
            wrdb_sb = pA_tail.tile([1, DOUT], F32, tag="wrdb_sb")
            sync.dma_start(wrdb_sb[:], wrdb[None, :])

            ytab_sb = pA_tail.tile([128, K // 128, DOUT], F32, tag="ytab_sb")
            for mc in range(K // 128):
                for nb in range(DOUT // 512):
                    psy = psA_m.tile([128, 512], F32, tag="psAm")
                    for dc in range(DM // 128):
                        pe.matmul(psy[:],
                                  rnT[:, dc, mc * 128:(mc + 1) * 128],
                                  wrd_sb[:, dc, nb * 512:(nb + 1) * 512],
                                  start=(dc == 0), stop=False)
                    pe.matmul(psy[:],
                              ones_sb.ap()[0:1, mc * 128:(mc + 1) * 128],
                              wrdb_sb[0:1, nb * 512:(nb + 1) * 512],
                              start=False, stop=True)
                    act.copy(ytab_sb[:, mc, nb * 512:(nb + 1) * 512], psy[:])
            sync.dma_start(
                ytab_dram[:, 0:DOUT].rearrange("(c p) d -> p c d", p=128), ytab_sb[:])
            # identity columns so the same gather also produces the one-hot modes
            import os as _os
            if _os.environ.get("KSTAGE", "full") != "noid":
                sync.dma_start(ytab_dram[:, DOUT:DOUT + K], iden256)
            if debug_outputs:
                sync.dma_start(dbg_ytab.rearrange("(c p) d -> p c d", p=128),
                               ytab_sb[:])
            pA_tail.release()

        # =====================================================================
        # Phase B: h^T = relu(x @ Wtr + b)^T, logits, sel tables
        # =====================================================================
        with tc.tile_pool(name="pB_w", bufs=1) as pB_w, \
             tc.tile_pool(name="pB_wt", bufs=8) as pB_wt, \
             tc.tile_pool(name="pB_x", bufs=2) as pB_x, \
             tc.tile_pool(name="pB_xt", bufs=1) as pB_xt, \
             tc.tile_pool(name="pB_h", bufs=1) as pB_h, \
             tc.tile_pool(name="pB_eps", bufs=2) as pB_eps, \
             tc.tile_pool(name="psB", bufs=2, space="PSUM") as psB, \
             tc.tile_pool(name="psB_t", bufs=2, space="PSUM") as psB_t, \
             tc.tile_pool(name="psB_l", bufs=2, space="PSUM") as psB_l:

            wtrb_sb = pB_w.tile([128, H // 128], F32, tag="wtrb_sb")
            sync.dma_start(wtrb_sb[:], wtrb.rearrange("(c p) -> p c", p=128))
            wms_sb = pB_w.tile([128, H // 128, K], F32, tag="wms_sb")
            sync.dma_start(wms_sb[:], wms.rearrange("(c p) k -> p c k", p=128))
            wmsb_sb = pB_w.tile([1, K], F32, tag="wmsb_sb")
            sync.dma_start(wmsb_sb[:], wmsb[None, :])

            # scan seed from prev_mode
            sync.dma_start(pm_sb.ap(), prev_mode)
            vec.max_with_indices(pmv.ap(), pmi.ap(), pm_sb.ap())
            vec.tensor_copy(jin.ap(), pmi.ap()[0:1, 0:1])
            scanA = nc.sync.alloc_register("scanA")
            sync.reg_load(scanA, jin.ap())

            for G in range(NGRP):
                # ---- x^T for this S-group via PE transpose ----
                xT = pB_xt.tile([128, DIN // 128, SGRP], F32, tag="xT")
                for sc in range(SGRP // 128):
                    xn = pB_x.tile([128, DIN], F32, tag="xn")
                    sync.dma_start(
                        xn[:], x[(G * SGRP + sc * 128):(G * SGRP + (sc + 1) * 128), :])
                    for dc in range(DIN // 128):
                        pt = psB_t.tile([128, 128], F32, tag="ptB")
                        pe.transpose(pt[:], xn[:, dc * 128:(dc + 1) * 128],
                                     iden_sb.ap())
                        act.copy(xT[:, dc, sc * 128:(sc + 1) * 128], pt[:])

                # ---- mm1 (fp32): h^T chunks; Wtr tiles streamed ----
                hT = pB_h.tile([128, H // 128, SGRP], F32, tag="hT")
                for m in range(H // 128):
                    ph = psB.tile([128, SGRP], F32, tag="ph")
                    for k in range(DIN // 128):
                        wt = pB_wt.tile([128, 128], F32, tag="wt")
                        sync.dma_start(
                            wt[:], wtr[k * 128:(k + 1) * 128, m * 128:(m + 1) * 128])
                        for nb in range(SGRP // 512):
                            pe.matmul(ph[:, nb * 512:(nb + 1) * 512],
                                      wt[:],
                                      xT[:, k, nb * 512:(nb + 1) * 512],
                                      start=(k == 0), stop=(k == DIN // 128 - 1))
                    act.activation(hT[:, m, :], ph[:], AF.Relu,
                                   bias=wtrb_sb[:, m:m + 1])

                # ---- mm2 (fp32) + epilogue per 128-row S chunk ----
                for mh in range(SGRP // 128):
                    c = G * (SGRP // 128) + mh          # global chunk id
                    pl = psB_l.tile([128, K], F32, tag="pl")
                    for k in range(H // 128):
                        pe.matmul(pl[:], hT[:, k, mh * 128:(mh + 1) * 128],
                                  wms_sb[:, k, :], start=(k == 0), stop=False)
                    pe.matmul(pl[:], ones_sb.ap()[0:1, 0:128], wmsb_sb[0:1, :],
                              start=False, stop=True)

                    lg = pB_eps.tile([128, K], F32, tag="lg")
                    vec.tensor_copy(lg[:], pl[:])
                    vmax = pB_eps.tile([128, 8], F32, tag="vmax")
                    aidx = pB_eps.tile([128, 8], U32, tag="aidx")
                    vec.max_with_indices(vmax[:], aidx[:], lg[:])
                    thr = pB_eps.tile([128, 1], F32, tag="thr")
                    vec.tensor_scalar_add(thr[:], vmax[:, 0:1], -STICKINESS)
                    af = pB_eps.tile([128, 1], F32, tag="af")
                    vec.tensor_copy(af[:], aidx[:, 0:1])
                    abase = pB_eps.tile([128, 1], F32, tag="abase")
                    vec.tensor_tensor(abase[:], af[:], base_sb.ap(), ALU.add)
                    stay = pB_eps.tile([128, K], F32, tag="stay")
                    vec.tensor_scalar(stay[:], lg[:], thr[:], None, ALU.is_ge)
                    d = pB_eps.tile([128, K], F32, tag="d")
                    vec.tensor_scalar(d[:], iota_sb.ap(), af[:], None, ALU.subtract)
                    vec.tensor_tensor(d[:], d[:], stay[:], ALU.mult)
                    vec.tensor_scalar(d[:], d[:], abase[:], None, ALU.add)
                    vec.tensor_copy(sel_nat.ap()[:, c * K:(c + 1) * K], d[:])
                    # redistribute into the ping-pong scan buffers, then emit
                    # this chunk's 128 scan steps (trace order drives Tile's
                    # RAW/WAR deps, pipelining DMA of chunk c+1 against them)
                    for hh in range(2):
                        sync.dma_start(
                            scanbuf[hh].ap(),
                            sel_nat.ap()[hh * SCH:(hh + 1) * SCH,
                                         c * K:(c + 1) * K])
                    for t in range(c * CHUNK, (c + 1) * CHUNK):
                        b = (t // SCH) % 2
                        sync.reg_load(scanA, bass.AP(scanbuf[b], scanA,
                                                     [[SCH * K, 1], [1, 1]]))
                        sync.reg_save(traj1.ap()[0:1, t:t + 1], scanA)

            # =================================================================
            # Gather indices: DRAM round-trip reshapes traj1 [1, S] to
            # trajB[p, c] = traj[16p + c] (both DMAs contiguous); then
            # j = trajB & 255, PE-transpose to the [16, S/16] wrapped layout
            # (slot i at [i%16, i//16]), replicate x8.
            # =================================================================
            sync.dma_start(trajd, traj1.ap())
            sync.dma_start(trajB.ap(),
                           trajd.rearrange("a (p c) -> p (a c)", p=128))
            vec.tensor_scalar(jv.ap(), trajB.ap(), 255, None, ALU.bitwise_and)
            vec.tensor_copy(jfB.ap(), jv.ap())
            ptj = psB_l.tile([16, 128], F32, tag="pl")
            pe.transpose(ptj[:], jfB.ap(), iden_sb.ap())
            vec.tensor_copy(idx16.ap(), ptj[:])

        for rep in range(8):
            sync.dma_start(idx_sb.ap()[rep * 16:(rep + 1) * 16, :], idx16.ap())

        with tc.tile_pool(name="pB_out", bufs=1) as pB_out:
            # 512-index sub-gathers: 2048 descriptors in one shot overflow the
            # SWDGE ring on HW; split also pipelines the output DMAs
            ysb = pB_out.tile([128, NCHUNK, DOUT + K], F32, tag="ysb")
            GCH = 512
            for b in range(S // GCH):
                gi = gp.dma_gather(
                    ysb[:, b * (GCH // 128):(b + 1) * (GCH // 128), :],
                    ytab_dram,
                    idx_sb.ap()[:, b * (GCH // 16):(b + 1) * (GCH // 16)],
                    GCH, GCH, DOUT + K)
                add_dep_helper(gi.ins, lib_inst.ins,
                               reason="gather needs mlp library")
                sync.dma_start(
                    y_out[b * GCH // 128 * 128:(b + 1) * GCH, :]
                    .rearrange("(c p) d -> p c d", p=128),
                    ysb[:, b * (GCH // 128):(b + 1) * (GCH // 128), 0:DOUT])
                sync.dma_start(
                    modes_out[b * GCH:(b + 1) * GCH, :]
                    .rearrange("(c p) k -> p c k", p=128),
                    ysb[:, b * (GCH // 128):(b + 1) * (GCH // 128), DOUT:DOUT + K])

        if debug_outputs:
            sync.dma_start(dbg_traj, trajB.ap())
            sync.dma_start(dbg_sel, sel_nat.ap())
            sync.dma_start(dbg_jin, jin.ap())

    nc.compile()
    return nc


# ---------------------------------------------------------------------------
# host side
# ---------------------------------------------------------------------------
_NC_CACHE = {}


def _get_nc(debug_outputs=False):
    key = bool(debug_outputs)
    if key not in _NC_CACHE:
        _NC_CACHE[key] = build_kernel(debug_outputs=key)
    return _NC_CACHE[key]


def _consts():
    iota = np.broadcast_to(np.arange(K, dtype=np.float32), (128, K)).copy()
    basecol = (256.0 * ((np.arange(128, dtype=np.float32) + 1) % SCH)).reshape(128, 1)
    iden = np.eye(128, dtype=np.float32)
    iden256 = np.eye(K, dtype=np.float32)
    return iota, basecol, iden, iden256


def make_in_maps(inp: dict):
    x = np.asarray(inp["x"], dtype=np.float32)
    prev_mode = np.asarray(inp["prev_mode"], dtype=np.float32)
    iota, basecol, iden, iden256 = _consts()
    shared = dict(
        Wtr_w=inp["Wtr_w"], Wtr_b=inp["Wtr_b"], Wms_w=inp["Wms_w"],
        Wms_b=inp["Wms_b"], M=inp["M"], g=inp["g"], Wrd_w=inp["Wrd_w"],
        Wrd_b=inp["Wrd_b"], bank_keys=inp["bank_keys"], bank_vals=inp["bank_vals"],
        bank_used=inp["bank_used"], iota_f=iota, basecol=basecol, iden=iden,
        iden256=iden256,
    )
    shared = {k: np.ascontiguousarray(np.asarray(v), dtype=np.float32)
              for k, v in shared.items()}
    in_maps = []
    for b in range(NCORES):
        m = dict(shared)
        m["x"] = np.ascontiguousarray(x[b])
        m["prev_mode"] = np.ascontiguousarray(prev_mode[b:b + 1])
        in_maps.append(m)
    return in_maps


def kernel(**inputs):
    inp = {k: np.asarray(v) for k, v in inputs.items()}
    in_maps = make_in_maps(inp)
    from concourse.bass_utils import run_bass_kernel_spmd
    nc = _get_nc()
    res = run_bass_kernel_spmd(nc, in_maps, list(range(NCORES)))
    y = np.stack([r["y"] for r in res.results], axis=0)
    modes = np.stack([r["modes"] for r in res.results], axis=0)
    return y, modes


# revision 66
# speedup vs baseline: 1.0385x; 1.0385x over previous
"""Trainium2 Bass kernel for nn_FRNNPathB (scatter_memory).

Strategy
--------
Data-parallel over batch B=8 across 8 NeuronCores (one batch element per core).

Key algorithmic observation: after the sequential mode scan, every downstream
quantity (mem, bank attention read, RMS norm, output projection) depends on the
token only through its mode index j in [0, 256). So the whole post-scan network
collapses to a per-mode table y_table[256, DOUT] computed once per core, and
y[s] = y_table[j_s] is a row gather. The bank attention (B*S*BANK*DM work in the
reference) is computed for 256 rows instead of 16384.

Per-core pipeline:
  Phase A (table): l2-normalize M rows and bank_keys, scores = qn @ kn^T * 4,
    attn = exp(scores) (cosine scores are bounded by +-4, no max-sub needed),
    bank = (attn @ (used*vals)) / (attn @ used), r = M + bank, RMS-norm with g
    folded into Wrd rows, y_table = rn @ Wrd + b  -> DRAM scratch.
  Phase B (logits): h^T = relu(Wtr^T x^T + b) and logits = h @ Wms + b in true
    fp32 on the PE (the argmax/stickiness decisions need fp32-level accuracy;
    measured min decision margin is ~2e-6 while fp32 cross-impl noise is ~3e-7).
  Scan: logits reduce to (max v_t, argmax a_t, staymask_t = lg_t >= v_t - 0.1).
    j_t = j_{t-1} if staymask[t, j_{t-1}] else a_t.  We precompute, in int16,
    sel[t, k] = 256*((t+1)%64) + (k if staymask[t,k] else a_t), sliced into 32
    tensors of 64 steps ([128, 128] i16, linear offset 256*(t%64)+j), so the
    scan is a pure pointer chase: one dependent reg_load per step on the Pool
    sequencer (plus a fire-and-forget reg_save of the trajectory). The scan and
    its buffer refills live on Pool so SP's in-order DMA-issue stream never
    blocks behind stalled scan steps (which would starve the PE of weights).
  Outputs: modes[t] = (iota == j_t), y rows gathered from y_table via dma_gather.
"""
import sys

sys.path.insert(0, "/opt/trn_rl_repo")

import numpy as np
from contextlib import ExitStack

import concourse.bacc as bacc
import concourse.bass as bass
import concourse.mybir as mybir
import concourse.tile as tile
from concourse import library_config
from concourse.tile_rust import add_dep_helper

B, S, DIN, H, K, DM, DOUT, BANK = 8, 2048, 1024, 2048, 256, 512, 1024, 4096
STICKINESS = 0.1
BANK_SCALE = 4.0
RMS_EPS = 1e-6

NCORES = 8
SGRP = 512                     # S columns per matmul-1 group
NGRP = S // SGRP               # 2
CHUNK = 128                    # logits chunk (= partition count per S tile)
NCHUNK = S // CHUNK            # 16
SCH = 64                       # scan steps per scan tensor
NSCH = S // SCH                # 32
F32 = mybir.dt.float32
F32R = mybir.dt.float32r
I32 = mybir.dt.int32
I16 = mybir.dt.int16
U32 = mybir.dt.uint32
AF = mybir.ActivationFunctionType
ALU = mybir.AluOpType


def build_kernel(debug_outputs: bool = False):
    """Build the single-core Bass program (SPMD across 8 cores)."""
    nc = bacc.Bacc("TRN2", target_bir_lowering=False, debug=False)

    # ---- external inputs (per core) ----
    x = nc.dram_tensor("x", [S, DIN], F32, kind="ExternalInput").ap()
    prev_mode = nc.dram_tensor("prev_mode", [1, K], F32, kind="ExternalInput").ap()
    wtr = nc.dram_tensor("Wtr_w", [DIN, H], F32, kind="ExternalInput").ap()
    wtrb = nc.dram_tensor("Wtr_b", [H], F32, kind="ExternalInput").ap()
    wms = nc.dram_tensor("Wms_w", [H, K], F32, kind="ExternalInput").ap()
    wmsb = nc.dram_tensor("Wms_b", [K], F32, kind="ExternalInput").ap()
    M_in = nc.dram_tensor("M", [K, DM], F32, kind="ExternalInput").ap()
    g_in = nc.dram_tensor("g", [DM], F32, kind="ExternalInput").ap()
    wrd = nc.dram_tensor("Wrd_w", [DM, DOUT], F32, kind="ExternalInput").ap()
    wrdb = nc.dram_tensor("Wrd_b", [DOUT], F32, kind="ExternalInput").ap()
    bkeys = nc.dram_tensor("bank_keys", [BANK, DM], F32, kind="ExternalInput").ap()
    bvals = nc.dram_tensor("bank_vals", [BANK, DM], F32, kind="ExternalInput").ap()
    bused = nc.dram_tensor("bank_used", [BANK], F32, kind="ExternalInput").ap()
    # host-provided constants (shape-derived only)
    iota_f = nc.dram_tensor("iota_f", [128, K], F32, kind="ExternalInput").ap()
    basecol = nc.dram_tensor("basecol", [128, 1], F32, kind="ExternalInput").ap()
    iden = nc.dram_tensor("iden", [128, 128], F32, kind="ExternalInput").ap()
    iden256 = nc.dram_tensor("iden256", [K, K], F32, kind="ExternalInput").ap()

    # ---- external outputs (per core) ----
    y_out = nc.dram_tensor("y", [S, DOUT], F32, kind="ExternalOutput").ap()
    modes_out = nc.dram_tensor("modes", [S, K], F32, kind="ExternalOutput").ap()
    if debug_outputs:
        dbg_traj = nc.dram_tensor("dbg_traj", [1, S], I32,
                                  kind="ExternalOutput").ap()
        dbg_ytab = nc.dram_tensor("dbg_ytab", [K, DOUT], F32, kind="ExternalOutput").ap()
        dbg_sel = nc.dram_tensor("dbg_sel", [128, NCHUNK * K], I16,
                                 kind="ExternalOutput").ap()
        dbg_jin = nc.dram_tensor("dbg_jin", [1, 1], I32, kind="ExternalOutput").ap()

    # DRAM scratch: per-mode [y_table | I256] rows for the output gather
    ytab_dram = nc.dram_tensor("ytab_scratch", [K, DOUT + K], F32).ap()
    trajd = nc.dram_tensor("traj_scratch", [1, S], I32).ap()

    with tile.TileContext(nc) as tc, ExitStack() as ctx:
        sync, vec, act, pe, gp = nc.sync, nc.vector, nc.scalar, nc.tensor, nc.gpsimd

        lib_inst = gp.load_library(library_config.mlp)

        # ---------------- small persistent constants ----------------
        iota_sb = nc.alloc_sbuf_tensor("iota_sb", [128, K], F32)
        sync.dma_start(iota_sb.ap(), iota_f)
        base_sb = nc.alloc_sbuf_tensor("base_sb", [128, 1], F32)
        sync.dma_start(base_sb.ap(), basecol)
        iden_sb = nc.alloc_sbuf_tensor("iden_sb", [128, 128], F32)
        sync.dma_start(iden_sb.ap(), iden)
        ones_sb = nc.alloc_sbuf_tensor("ones_sb", [1, K], F32)
        vec.memset(ones_sb.ap(), 1.0)

        # all raw (non-pool) SBUF tensors up front, so Tile pools never
        # overlap their address range
        sel_nat = nc.alloc_sbuf_tensor("sel_nat", [128, NCHUNK * K], I16)
        # ping-pong single-partition scan buffers: 64 steps each; the dynamic
        # reg_load offset must stay within one partition row
        scanbuf = [nc.alloc_sbuf_tensor(f"scanbuf{b}", [1, SCH * K], I16)
                   for b in range(2)]
        traj1 = nc.alloc_sbuf_tensor("traj1", [1, S], I32)
        trajB = nc.alloc_sbuf_tensor("trajB", [32, (S // 512) * 16], I32)
        pm_sb = nc.alloc_sbuf_tensor("pm_sb", [1, K], F32)
        pmv = nc.alloc_sbuf_tensor("pmv", [1, 8], F32)
        pmi = nc.alloc_sbuf_tensor("pmi", [1, 8], U32)
        jin = nc.alloc_sbuf_tensor("jin", [1, 1], I32)
        jv = nc.alloc_sbuf_tensor("jv", [32, (S // 512) * 16], I32)
        jfB = nc.alloc_sbuf_tensor("jfB", [32, (S // 512) * 16], F32)
        idx16 = nc.alloc_sbuf_tensor("idx16", [16, 128], I16)
        idx_sb = nc.alloc_sbuf_tensor("idx_sb", [128, S // 16], I16)

        # =====================================================================
        # Phase A: the 256-row mode table
        # =====================================================================
        with tc.tile_pool(name="pA_small", bufs=2) as pA_small, \
             tc.tile_pool(name="pA_keep", bufs=1) as pA, \
             tc.tile_pool(name="psA_t", bufs=2, space="PSUM") as psA_t, \
             tc.tile_pool(name="psA_m", bufs=2, space="PSUM") as psA_m, \
             tc.tile_pool(name="psA_d", bufs=2, space="PSUM") as psA_d:

            def newton_rsqrt(col_out, col_in, n):
                """1/sqrt via ACT Sqrt + DVE reciprocal + one Newton polish."""
                s0 = pA_small.tile([128, n], F32, tag="nr_s0")
                act.activation(s0[:], col_in, AF.Sqrt)
                y0 = pA_small.tile([128, n], F32, tag="nr_y0")
                vec.reciprocal(y0[:], s0[:])
                t1 = pA_small.tile([128, n], F32, tag="nr_t1")
                vec.tensor_tensor(t1[:], y0[:], s0[:], ALU.mult)
                vec.tensor_scalar(t1[:], t1[:], -1.0, 2.0, ALU.mult, ALU.add)
                vec.tensor_tensor(col_out, y0[:], t1[:], ALU.mult)

            # --- M rows -> qn -> qn^T ---
            m_sb = pA.tile([128, K // 128, DM], F32, tag="m_sb")
            sync.dma_start(m_sb[:], M_in.rearrange("(c p) d -> p c d", p=128))
            msq = pA_small.tile([128, K // 128], F32, tag="msq")
            scr = pA_small.tile([128, DM], F32, tag="scrA")
            for c in range(K // 128):
                act.activation(scr[:], m_sb[:, c, :], AF.Square,
                               accum_out=msq[:, c:c + 1])
            mrsq = pA_small.tile([128, K // 128], F32, tag="mrsq")
            newton_rsqrt(mrsq[:], msq[:], K // 128)
            qnT = pA.tile([128, DM // 128, K], F32, tag="qnT")
            with tc.tile_pool(name="pA_qn", bufs=1) as pA_qn:
                qn_sb = pA_qn.tile([128, K // 128, DM], F32, tag="qn_sb")
                for c in range(K // 128):
                    vec.tensor_scalar_mul(qn_sb[:, c, :], m_sb[:, c, :],
                                          mrsq[:, c:c + 1])
                for c in range(K // 128):
                    for dc in range(DM // 128):
                        pt = psA_t.tile([128, 128], F32, tag="ptA")
                        pe.transpose(pt[:], qn_sb[:, c, dc * 128:(dc + 1) * 128],
                                     iden_sb.ap())
                        act.copy(qnT[:, dc, c * 128:(c + 1) * 128], pt[:])

            # --- bank_keys streamed -> kn^T; attn^T = exp(4 qn kn^T)^T built
            #     strip-by-strip (the full [K, BANK] attn is never materialized)
            pA_at = tc.alloc_tile_pool(name="pA_at", bufs=1)
            attnT = pA_at.tile([128, BANK // 128, K], F32, tag="attnT")
            four_sb = pA_small.tile([128, 1], F32, tag="four")
            vec.memset(four_sb[:], BANK_SCALE)
            HB = BANK // 2
            for half in range(2):
                with tc.tile_pool(name=f"pA_knT{half}", bufs=1) as pA_knT, \
                     tc.tile_pool(name=f"pA_bks{half}", bufs=3) as pA_bks:
                    knT = pA_knT.tile([128, DM // 128, HB], F32, tag="knT")
                    for c in range(HB // 128):
                        gc = half * (HB // 128) + c
                        bkc = pA_bks.tile([128, DM], F32, tag="bkc")
                        sync.dma_start(bkc[:], bkeys[gc * 128:(gc + 1) * 128, :])
                        ksq = pA_small.tile([128, 1], F32, tag="ksq")
                        act.activation(scr[:], bkc[:], AF.Square, accum_out=ksq[:])
                        krs = pA_small.tile([128, 1], F32, tag="krs")
                        newton_rsqrt(krs[:], ksq[:], 1)
                        vec.tensor_scalar_mul(bkc[:], bkc[:], krs[:])
                        for dc in range(DM // 128):
                            pt = psA_t.tile([128, 128], F32, tag="ptA")
                            pe.transpose(pt[:], bkc[:, dc * 128:(dc + 1) * 128],
                                         iden_sb.ap())
                            act.copy(knT[:, dc, c * 128:(c + 1) * 128], pt[:])

                    for mc in range(K // 128):
                        for nb in range(HB // 512):
                            ps = psA_m.tile([128, 512], F32, tag="psAm")
                            for dc in range(DM // 128):
                                pe.matmul(
                                    ps[:],
                                    qnT[:, dc, mc * 128:(mc + 1) * 128],
                                    knT[:, dc, nb * 512:(nb + 1) * 512],
                                    start=(dc == 0), stop=(dc == DM // 128 - 1))
                            strip = pA_bks.tile([128, 512], F32, tag="strip")
                            act.activation(strip[:], ps[:], AF.Exp, scale=four_sb[:])
                            for tb in range(4):
                                bc = half * (HB // 128) + nb * 4 + tb
                                pt = psA_t.tile([128, 128], F32, tag="ptA")
                                pe.transpose(pt[:], strip[:, tb * 128:(tb + 1) * 128],
                                             iden_sb.ap())
                                act.copy(attnT[:, bc, mc * 128:(mc + 1) * 128], pt[:])

            # --- bank read streamed over bank_vals; r = M + bank ---
            r_sb = pA.tile([128, K // 128, DM], F32, tag="r_sb")
            with tc.tile_pool(name="pA_bvstrip", bufs=3) as pA_bvs:
                bv_tiles = []
                for bc in range(BANK // 128):
                    bvc = pA_bvs.tile([128, DM + 1], F32, tag="bvc")
                    sync.dma_start(bvc[:, 0:DM], bvals[bc * 128:(bc + 1) * 128, :])
                    sync.dma_start(bvc[:, DM:DM + 1],
                                   bused[bc * 128:(bc + 1) * 128][:, None])
                    vec.tensor_scalar_mul(bvc[:, 0:DM], bvc[:, 0:DM],
                                          bvc[:, DM:DM + 1])
                    bv_tiles.append(bvc)

                for mc in range(K // 128):
                    psn = psA_m.tile([128, 512], F32, tag="psAm")
                    psd = psA_d.tile([128, 8], F32, tag="psAd")
                    for bc in range(BANK // 128):
                        bvc = bv_tiles[bc]
                        last = bc == BANK // 128 - 1
                        pe.matmul(psn[:],
                                  attnT[:, bc, mc * 128:(mc + 1) * 128],
                                  bvc[:, 0:DM],
                                  start=(bc == 0), stop=last)
                        pe.matmul(psd[:, 0:1],
                                  attnT[:, bc, mc * 128:(mc + 1) * 128],
                                  bvc[:, DM:DM + 1],
                                  start=(bc == 0), stop=last)
                    den = pA_small.tile([128, 1], F32, tag="denA")
                    rec = pA_small.tile([128, 1], F32, tag="recA")
                    act.copy(den[:], psd[:, 0:1])
                    vec.reciprocal(rec[:], den[:])
                    t2 = pA_small.tile([128, 1], F32, tag="recT")
                    vec.tensor_tensor(t2[:], rec[:], den[:], ALU.mult)
                    vec.tensor_scalar(t2[:], t2[:], -1.0, 2.0, ALU.mult, ALU.add)
                    vec.tensor_tensor(rec[:], rec[:], t2[:], ALU.mult)
                    bank_t = pA_small.tile([128, DM], F32, tag="bankA")
                    vec.tensor_scalar_mul(bank_t[:], psn[:], rec[:])
                    vec.tensor_tensor(r_sb[:, mc, :], m_sb[:, mc, :], bank_t[:],
                                      ALU.add)

            pA_at.release()

            # --- rms-norm of r; rn^T; y_table ---
            rsq2 = pA_small.tile([128, K // 128], F32, tag="rsq2")
            for mc in range(K // 128):
                act.activation(scr[:], r_sb[:, mc, :], AF.Square,
                               accum_out=rsq2[:, mc:mc + 1])
            vec.tensor_scalar(rsq2[:], rsq2[:], 1.0 / DM, RMS_EPS, ALU.mult, ALU.add)
            rms = pA_small.tile([128, K // 128], F32, tag="rms")
            newton_rsqrt(rms[:], rsq2[:], K // 128)
            for mc in range(K // 128):
                vec.tensor_scalar_mul(r_sb[:, mc, :], r_sb[:, mc, :],
                                      rms[:, mc:mc + 1])
            rnT = pA.tile([128, DM // 128, K], F32, tag="rnT")
            for mc in range(K // 128):
                for dc in range(DM // 128):
                    pt = psA_t.tile([128, 128], F32, tag="ptA")
                    pe.transpose(pt[:], r_sb[:, mc, dc * 128:(dc + 1) * 128],
                                 iden_sb.ap())
                    act.copy(rnT[:, dc, mc * 128:(mc + 1) * 128], pt[:])

            pA_tail = tc.alloc_tile_pool(name="pA_tail", bufs=1)
            wrd_sb = pA_tail.tile([128, DM // 128, DOUT], F32, tag="wrd_sb")
            sync.dma_start(wrd_sb[:], wrd.rearrange("(c p) d -> p c d", p=128))
            g_sb = pA_small.tile([128, DM // 128], F32, tag="g_sb")
            sync.dma_start(g_sb[:], g_in.rearrange("(c p) -> p c", p=128))
            for dc in range(DM // 128):
                vec.tensor_scalar_mul(wrd_sb[:, dc, :], wrd_sb[:, dc, :],
                                      g_sb[:, dc:dc + 1])
            wrdb_sb = pA_tail.tile([1, DOUT], F32, tag="wrdb_sb")
            sync.dma_start(wrdb_sb[:], wrdb[None, :])

            ytab_sb = pA_tail.tile([128, K // 128, DOUT], F32, tag="ytab_sb")
            for mc in range(K // 128):
                for nb in range(DOUT // 512):
                    psy = psA_m.tile([128, 512], F32, tag="psAm")
                    for dc in range(DM // 128):
                        pe.matmul(psy[:],
                                  rnT[:, dc, mc * 128:(mc + 1) * 128],
                                  wrd_sb[:, dc, nb * 512:(nb + 1) * 512],
                                  start=(dc == 0), stop=False)
                    pe.matmul(psy[:],
                              ones_sb.ap()[0:1, mc * 128:(mc + 1) * 128],
                              wrdb_sb[0:1, nb * 512:(nb + 1) * 512],
                              start=False, stop=True)
                    act.copy(ytab_sb[:, mc, nb * 512:(nb + 1) * 512], psy[:])
            sync.dma_start(
                ytab_dram[:, 0:DOUT].rearrange("(c p) d -> p c d", p=128), ytab_sb[:])
            # identity columns so the same gather also produces the one-hot modes
            sync.dma_start(ytab_dram[:, DOUT:DOUT + K], iden256)
            if debug_outputs:
                sync.dma_start(dbg_ytab.rearrange("(c p) d -> p c d", p=128),
                               ytab_sb[:])
            pA_tail.release()

        # =====================================================================
        # Phase B: h^T = relu(x @ Wtr + b)^T, logits, sel tables
        # =====================================================================
        with tc.tile_pool(name="pB_w", bufs=1) as pB_w, \
             tc.tile_pool(name="pB_wt", bufs=6) as pB_wt, \
             tc.tile_pool(name="pB_x", bufs=2) as pB_x, \
             tc.tile_pool(name="pB_xt", bufs=1) as pB_xt, \
             tc.tile_pool(name="pB_h", bufs=1) as pB_h, \
             tc.tile_pool(name="pB_eps", bufs=2) as pB_eps, \
             tc.tile_pool(name="psB", bufs=2, space="PSUM") as psB, \
             tc.tile_pool(name="psB_t", bufs=2, space="PSUM") as psB_t, \
             tc.tile_pool(name="psB_l", bufs=2, space="PSUM") as psB_l:

            wtrb_sb = pB_w.tile([128, H // 128], F32, tag="wtrb_sb")
            sync.dma_start(wtrb_sb[:], wtrb.rearrange("(c p) -> p c", p=128))
            wms_sb = pB_w.tile([128, H // 128, K], F32, tag="wms_sb")
            sync.dma_start(wms_sb[:], wms.rearrange("(c p) k -> p c k", p=128))
            wmsb_sb = pB_w.tile([1, K], F32, tag="wmsb_sb")
            sync.dma_start(wmsb_sb[:], wmsb[None, :])

            # scan seed from prev_mode
            sync.dma_start(pm_sb.ap(), prev_mode)
            vec.max_with_indices(pmv.ap(), pmi.ap(), pm_sb.ap())
            vec.tensor_copy(jin.ap(), pmi.ap()[0:1, 0:1])
            # scan runs on the otherwise-idle Pool sequencer: putting it on SP
            # would queue every later dma_start issue behind stalled scan
            # steps and starve the PE of weight tiles
            scanA = nc.gpsimd.alloc_register("scanA")
            gp.reg_load(scanA, jin.ap())

            for G in range(NGRP):
                # ---- x^T for this S-group via PE transpose ----
                xT = pB_xt.tile([128, DIN // 128, SGRP], F32, tag="xT")
                for sc in range(SGRP // 128):
                    xn = pB_x.tile([128, DIN], F32, tag="xn")
                    sync.dma_start(
                        xn[:], x[(G * SGRP + sc * 128):(G * SGRP + (sc + 1) * 128), :])
                    for dc in range(DIN // 128):
                        pt = psB_t.tile([128, 128], F32, tag="ptB")
                        pe.transpose(pt[:], xn[:, dc * 128:(dc + 1) * 128],
                                     iden_sb.ap())
                        act.copy(xT[:, dc, sc * 128:(sc + 1) * 128], pt[:])

                # ---- mm1 (fp32): h^T chunks; Wtr tiles streamed ----
                hT = pB_h.tile([128, H // 128, SGRP], F32, tag="hT")
                for m in range(H // 128):
                    ph = psB.tile([128, SGRP], F32, tag="ph")
                    for k in range(DIN // 128):
                        wt = pB_wt.tile([128, 128], F32, tag="wt")
                        sync.dma_start(
                            wt[:], wtr[k * 128:(k + 1) * 128, m * 128:(m + 1) * 128])
                        for nb in range(SGRP // 512):
                            pe.matmul(ph[:, nb * 512:(nb + 1) * 512],
                                      wt[:],
                                      xT[:, k, nb * 512:(nb + 1) * 512],
                                      start=(k == 0), stop=(k == DIN // 128 - 1))
                    act.activation(hT[:, m, :], ph[:], AF.Relu,
                                   bias=wtrb_sb[:, m:m + 1])

                # ---- mm2 (fp32) + epilogue per 128-row S chunk ----
                for mh in range(SGRP // 128):
                    c = G * (SGRP // 128) + mh          # global chunk id
                    pl = psB_l.tile([128, K], F32, tag="pl")
                    for k in range(H // 128):
                        pe.matmul(pl[:], hT[:, k, mh * 128:(mh + 1) * 128],
                                  wms_sb[:, k, :], start=(k == 0), stop=False)
                    pe.matmul(pl[:], ones_sb.ap()[0:1, 0:128], wmsb_sb[0:1, :],
                              start=False, stop=True)

                    lg = pB_eps.tile([128, K], F32, tag="lg")
                    vec.tensor_copy(lg[:], pl[:])
                    vmax = pB_eps.tile([128, 8], F32, tag="vmax")
                    aidx = pB_eps.tile([128, 8], U32, tag="aidx")
                    vec.max_with_indices(vmax[:], aidx[:], lg[:])
                    thr = pB_eps.tile([128, 1], F32, tag="thr")
                    vec.tensor_scalar_add(thr[:], vmax[:, 0:1], -STICKINESS)
                    af = pB_eps.tile([128, 1], F32, tag="af")
                    vec.tensor_copy(af[:], aidx[:, 0:1])
                    abase = pB_eps.tile([128, 1], F32, tag="abase")
                    vec.tensor_tensor(abase[:], af[:], base_sb.ap(), ALU.add)
                    stay = pB_eps.tile([128, K], F32, tag="stay")
                    vec.tensor_scalar(stay[:], lg[:], thr[:], None, ALU.is_ge)
                    d = pB_eps.tile([128, K], F32, tag="d")
                    vec.tensor_scalar(d[:], iota_sb.ap(), af[:], None, ALU.subtract)
                    vec.tensor_tensor(d[:], d[:], stay[:], ALU.mult)
                    vec.tensor_scalar(d[:], d[:], abase[:], None, ALU.add)
                    vec.tensor_copy(sel_nat.ap()[:, c * K:(c + 1) * K], d[:])
                    # redistribute into the ping-pong scan buffers from the
                    # POOL queue: same-engine program order makes the WAR
                    # (refill vs previous phase's reads) free, and SP's
                    # in-order DMA stream never blocks on scan progress
                    for hh in range(2):
                        gp.dma_start(
                            scanbuf[hh].ap(),
                            sel_nat.ap()[hh * SCH:(hh + 1) * SCH,
                                         c * K:(c + 1) * K])
                    for t in range(c * CHUNK, (c + 1) * CHUNK):
                        b = (t // SCH) % 2
                        gp.reg_load(scanA, bass.AP(scanbuf[b], scanA,
                                                   [[SCH * K, 1], [1, 1]]))
                        gp.reg_save(traj1.ap()[0:1, t:t + 1], scanA)

            # =================================================================
            # Per-512-step blocks, pipelined behind the scan: DRAM round-trip
            # reshapes traj1 slice to trajB[p, c] = traj[16p + c] (both DMAs
            # contiguous), j = trajB & 255, PE-transpose to the wrapped [16, n]
            # index layout (slot i at [i%16, i//16]) replicated x8, then the
            # sub-gather (<=512 descriptors per call: 2048 in one shot
            # overflows the SWDGE ring on HW) and the output DMAs.
            # =================================================================
            GCH = 512
            PB = GCH // 16                     # trajB rows per block
            with tc.tile_pool(name="pB_out", bufs=2) as pB_out:
                for b in range(S // GCH):
                    r0 = b * PB
                    cc = b * 16
                    sync.dma_start(trajd[:, b * GCH:(b + 1) * GCH],
                                   traj1.ap()[:, b * GCH:(b + 1) * GCH])
                    sync.dma_start(
                        trajB.ap()[:, cc:cc + 16],
                        trajd[:, b * GCH:(b + 1) * GCH]
                        .rearrange("a (p c) -> p (a c)", p=PB))
                    vec.tensor_scalar(jv.ap()[:, cc:cc + 16],
                                      trajB.ap()[:, cc:cc + 16], 255, None,
                                      ALU.bitwise_and)
                    vec.tensor_copy(jfB.ap()[:, cc:cc + 16], jv.ap()[:, cc:cc + 16])
                    ptj = psB_l.tile([16, PB], F32, tag="pl")
                    pe.transpose(ptj[:], jfB.ap()[:, cc:cc + 16],
                                 iden_sb.ap()[0:PB, 0:PB])
                    vec.tensor_copy(idx16.ap()[:, r0:r0 + PB], ptj[:])
                    for rep in range(8):
                        sync.dma_start(
                            idx_sb.ap()[rep * 16:(rep + 1) * 16, r0:r0 + PB],
                            idx16.ap()[:, r0:r0 + PB])
                    ysb = pB_out.tile([128, GCH // 128, DOUT + K], F32, tag="ysb")
                    gi = gp.dma_gather(
                        ysb[:],
                        ytab_dram,
                        idx_sb.ap()[:, b * (GCH // 16):(b + 1) * (GCH // 16)],
                        GCH, GCH, DOUT + K)
                    add_dep_helper(gi.ins, lib_inst.ins,
                                   reason="gather needs mlp library")
                    sync.dma_start(
                        y_out[b * GCH:(b + 1) * GCH, :]
                        .rearrange("(c p) d -> p c d", p=128),
                        ysb[:, :, 0:DOUT])
                    sync.dma_start(
                        modes_out[b * GCH:(b + 1) * GCH, :]
                        .rearrange("(c p) k -> p c k", p=128),
                        ysb[:, :, DOUT:DOUT + K])

        if debug_outputs:
            sync.dma_start(dbg_traj, traj1.ap())
            sync.dma_start(dbg_sel, sel_nat.ap())
            sync.dma_start(dbg_jin, jin.ap())

    nc.compile()
    return nc


# ---------------------------------------------------------------------------
# host side
# ---------------------------------------------------------------------------
_NC_CACHE = {}


def _get_nc(debug_outputs=False):
    key = bool(debug_outputs)
    if key not in _NC_CACHE:
        _NC_CACHE[key] = build_kernel(debug_outputs=key)
    return _NC_CACHE[key]


def _consts():
    iota = np.broadcast_to(np.arange(K, dtype=np.float32), (128, K)).copy()
    basecol = (256.0 * ((np.arange(128, dtype=np.float32) + 1) % SCH)).reshape(128, 1)
    iden = np.eye(128, dtype=np.float32)
    iden256 = np.eye(K, dtype=np.float32)
    return iota, basecol, iden, iden256


def make_in_maps(inp: dict):
    x = np.asarray(inp["x"], dtype=np.float32)
    prev_mode = np.asarray(inp["prev_mode"], dtype=np.float32)
    iota, basecol, iden, iden256 = _consts()
    shared = dict(
        Wtr_w=inp["Wtr_w"], Wtr_b=inp["Wtr_b"], Wms_w=inp["Wms_w"],
        Wms_b=inp["Wms_b"], M=inp["M"], g=inp["g"], Wrd_w=inp["Wrd_w"],
        Wrd_b=inp["Wrd_b"], bank_keys=inp["bank_keys"], bank_vals=inp["bank_vals"],
        bank_used=inp["bank_used"], iota_f=iota, basecol=basecol, iden=iden,
        iden256=iden256,
    )
    shared = {k: np.ascontiguousarray(np.asarray(v), dtype=np.float32)
              for k, v in shared.items()}
    in_maps = []
    for b in range(NCORES):
        m = dict(shared)
        m["x"] = np.ascontiguousarray(x[b])
        m["prev_mode"] = np.ascontiguousarray(prev_mode[b:b + 1])
        in_maps.append(m)
    return in_maps


def kernel(**inputs):
    inp = {k: np.asarray(v) for k, v in inputs.items()}
    in_maps = make_in_maps(inp)
    from concourse.bass_utils import run_bass_kernel_spmd
    nc = _get_nc()
    res = run_bass_kernel_spmd(nc, in_maps, list(range(NCORES)))
    y = np.stack([r["y"] for r in res.results], axis=0)
    modes = np.stack([r["modes"] for r in res.results], axis=0)
    return y, modes
